# revision 83
# baseline (speedup 1.0000x reference)
"""Trainium2 Bass kernel for nn_MultiHeadAttention_72378788872456.

Sharding (8 cores): core c handles batch b = c//4 and head group g = c%4
(heads 4g..4g+3).  Tensor-parallel on heads within each batch's 4-core
group; the only collective is a chunked 4-rank ReduceScatter after the
P_o contraction.

On-device layouts are all "transposed" so no device-side transposes are
needed (the host pre-transposes per-core shards, which is part of
sharding/unsharding):
  qT/kT/vT inputs: [d=128, d_tile, n]    (contraction dim d on partitions)
  q/k after proj+rope: per head-pair tiles [128 = 2*64 k-dims, n]
  scores S^T: [m, n] tiles; softmax denominator comes for free from a
  ones-column appended to the V stationary of the o^T matmul.
  o^T: [hv, n];  output projection emits natural [n, d] partials.

Data flows in 16-bit with f32 PSUM accumulation everywhere; 16-bit
matmuls run at full PE rate at any free-dim and halve all HBM traffic.
The q/k path (inputs, P_q/P_k, rope'd tiles, sin/cos maps) uses fp16 --
logits have sigma~8, so softmax exponentiation amplifies logit noise
~e^delta, and bf16's 8-bit mantissa there costs ~3e-2 rel error while
fp16 keeps it at ~6e-3.  The v/exp/output path uses bf16 (exp values
reach e^30, beyond fp16 range; errors there do not amplify).

Schedule: one startup loop streams k/v/q chunk-by-chunk (projections +
rope hide behind the input DMA), then 8 attention stages (chunk x pair)
run an ACT-bound S->exp->oT pipeline; each stage's softmax
normalization and the per-chunk output projection + ReduceScatter are
deferred into the NEXT stage's inner loop so no engine ever drains at
stage boundaries.
"""

import math
import os
import sys
import numpy as np

# ---------------------------------------------------------------- constants
B, N, M, D, H, K, V = 2, 2048, 2048, 1024, 16, 64, 64
MAX_WAVELENGTH = 10000.0
SCALE_FACTOR = 1.0
N_CORES = 8
HLOC = 4            # heads per core
PAIRS = HLOC // 2   # head-pairs per core
P = 128
FREE = 512          # matmul moving free-dim / n-chunk granularity

_COMPILED = {}      # dims -> (nc, meta)


def build_nc(n=N, m=M, d=D, n_cores=N_CORES, group_size=4, cast_bias=0.0,
             use_collective=True, reps=1, shared_maps=False, phase="all"):
    """Build the SPMD Bass program (identical on every core)."""
    import concourse.bass as bass
    import concourse.mybir as mybir
    import concourse.tile as tile
    from concourse import bacc

    dt = mybir.dt
    f32 = dt.float32
    bf16 = dt.bfloat16
    fp16 = dt.float16
    AF = mybir.ActivationFunctionType
    ALU = mybir.AluOpType

    DT = d // P            # d tiles (contraction steps) for projections
    NC4 = n // FREE        # n chunks
    MT = m // P            # m tiles
    MC4 = m // FREE        # m chunks
    NTPC = FREE // P       # n tiles per chunk (outproj stationaries)
    DC = d // FREE         # d chunks in outproj output
    RG = [list(range(g * group_size, (g + 1) * group_size))
          for g in range(n_cores // group_size)]

    nc = bacc.Bacc("TRN2", target_bir_lowering=False, debug=False,
                   num_devices=n_cores)

    # ------------------------------------------------ DRAM I/O declarations
    qT_d = nc.dram_tensor("qT", [P, DT, n], fp16, kind="ExternalInput").ap()
    kT_d = nc.dram_tensor("kT", [P, DT, m], fp16, kind="ExternalInput").ap()
    vT_d = nc.dram_tensor("vT", [P, DT, m], bf16, kind="ExternalInput").ap()
    pq_d = nc.dram_tensor("pq", [P, DT, 2 * P], fp16, kind="ExternalInput").ap()
    pk_d = nc.dram_tensor("pk", [P, DT, 2 * P], fp16, kind="ExternalInput").ap()
    pv_d = nc.dram_tensor("pv", [P, DT, 2 * P], bf16, kind="ExternalInput").ap()
    po_d = nc.dram_tensor("po", [P, PAIRS, d], bf16, kind="ExternalInput").ap()
    # positions pre-broadcast to all 128 partitions by the host, so the
    # rope phase y[p,f] = pos[f]*invt2pi[p] is a single DVE tensor_scalar
    # (no PE outer product, no Cody-Waite: the *2pi is folded into the
    # Sin activation's scale operand)
    # fp16: integer positions up to 2048 are exact, half the DMA bytes
    qpos_d = nc.dram_tensor("qposb", [P, n], fp16, kind="ExternalInput").ap()
    kpos_d = nc.dram_tensor("kposb", [P, m], fp16, kind="ExternalInput").ap()
    # rope consts [128, 2]: col0 invt/(2pi), col1 sign*2pi
    rc_d = nc.dram_tensor("ropec", [P, 2], f32, kind="ExternalInput").ap()
    if use_collective:
        out_d = nc.dram_tensor("out_part", [NC4, group_size * P // 4, d], bf16,
                               kind="ExternalOutput").ap()
    else:
        out_d = nc.dram_tensor("out_part", [NC4, FREE, d], bf16,
                               kind="ExternalOutput").ap()
    RS_ROWS = FREE // group_size  # 128 for group_size=4

    TWO_PI = 2.0 * math.pi
    INV_2PI = 1.0 / TWO_PI
    MAGIC = 1.5 * 2.0 ** 23       # fp32 round-to-nearest via add/sub

    def _trunc12(v):
        x = np.float32(v)
        u = x.view(np.uint32) & np.uint32(0xFFFFF000)
        return float(u.view(np.float32))

    CW1 = _trunc12(TWO_PI)
    CW2 = _trunc12(TWO_PI - CW1)
    CW3 = float(np.float32(TWO_PI - CW1 - CW2))

    with tile.TileContext(nc) as tc:
        with (
            tc.tile_pool(name="persist", bufs=1) as persist,
            tc.tile_pool(name="maps", bufs=4) as mapsp,
            tc.tile_pool(name="posfp", bufs=1) as posfp,
            tc.tile_pool(name="mtmp", bufs=2) as mtmp,
            tc.tile_pool(name="instream", bufs=2) as instream,
            tc.tile_pool(name="expp", bufs=4) as expp,
            tc.tile_pool(name="nrm", bufs=2) as nrm,
            tc.tile_pool(name="otn", bufs=3) as otnp,
            tc.tile_pool(name="stg", bufs=3) as stgp,
            tc.tile_pool(name="stps", bufs=2, space="PSUM") as stps,
            tc.tile_pool(name="otps", bufs=2, space="PSUM") as otps,
            tc.tile_pool(name="mmps", bufs=2, space="PSUM") as mmps,
            tc.tile_pool(name="dram", bufs=4, space="DRAM") as dramp,
        ):
            # ---------------------------------------------------- constants
            po_sb = persist.tile([P, PAIRS, d], bf16, tag="po")
            rc_sb = persist.tile([P, 2], f32, tag="rc")
            ebc_sb = persist.tile([P, P], bf16, tag="ebc")
            nc.sync.dma_start(rc_sb[:], rc_d[:, :])
            # normalization-broadcast stationary, built on-device:
            # row0 -> out parts 0:64, row32 -> out parts 64:128
            nc.vector.memset(ebc_sb[:], 0.0)
            nc.vector.memset(ebc_sb[0:1, 0:64], 1.0)
            nc.vector.memset(ebc_sb[32:33, 64:128], 1.0)
            # normalization moving tiles: rows other than 0/32 multiply
            # zero stationary entries, so a one-time zero fill suffices
            # (stale reciprocals on later reuses are harmless).
            rzring = [persist.tile([P, FREE], bf16, tag=f"rz{i}",
                                   name=f"rz{i}")
                      for i in range(2)]
            for rzt in rzring:
                nc.vector.memset(rzt[:], 0.0)
            invt2pi = rc_sb[:, 0:1]
            sgn2pi = rc_sb[:, 1:2]
            SWAP_MASK = [i ^ 1 for i in range(32)]

            def one_pass(rep):
                def make_maps_chunk(pos_d, c, tag):
                    """sin'/cos map tiles [P, FREE] for columns c*FREE..

                    y = pos*invt/(2pi) + quarter; round via the fp32 magic
                    number; r = y - round(y) is the phase mod 1 turn in
                    [-0.5, 0.5].  sin' = sin(r*2pi*sgn); with quarter=0.25
                    the cos bias cancels: cos = sin(r*2pi)."""
                    sl = slice(c * FREE, (c + 1) * FREE)
                    posb = posfp.tile([P, FREE], fp16, tag="posb",
                                      name=f"posb_{tag}{c}")
                    nc.sync.dma_start(posb[:], pos_d[:, sl])
                    sinm = mapsp.tile([P, FREE], fp16, tag="sinm",
                                      name=f"sin_{tag}{c}")
                    cosm = mapsp.tile([P, FREE], fp16, tag="cosm",
                                      name=f"cos_{tag}{c}")

                    def reduce_and_sin(out_ap, quarter, scale):
                        y = mtmp.tile([P, FREE], f32, tag="y")
                        nc.vector.tensor_scalar(y[:], posb[:], invt2pi,
                                                cast_bias + quarter,
                                                ALU.mult, ALU.add)
                        yf = mtmp.tile([P, FREE], f32, tag="yf")
                        nc.vector.tensor_scalar(yf[:], y[:], MAGIC, MAGIC,
                                                ALU.add, ALU.subtract)
                        r = mtmp.tile([P, FREE], f32, tag="r")
                        nc.vector.tensor_tensor(r[:], y[:], yf[:],
                                                ALU.subtract)
                        nc.scalar.activation(out_ap, r[:], AF.Sin,
                                             scale=scale)

                    reduce_and_sin(sinm[:], 0.0, sgn2pi)
                    reduce_and_sin(cosm[:], 0.25, TWO_PI)
                    return sinm, cosm

                def rope_from_ps(ps, sinm, cosm, name):
                    """ACT drains the projection PSUM to a fp16 SBUF copy
                    so the whole rope chain runs on DVE in the 2-byte fast
                    path; maps are fp16 for the same reason."""
                    psb = mtmp.tile([P, FREE], fp16, tag="psb",
                                    name=f"psb_{name}")
                    nc.scalar.copy(psb[:], ps[:])
                    t1 = mtmp.tile([P, FREE], fp16, tag="t1",
                                   name=f"t1_{name}")
                    nc.vector.tensor_tensor(t1[:], psb[:], cosm[:], ALU.mult)
                    xsw = mtmp.tile([P, FREE], fp16, tag="xsw",
                                    name=f"xsw_{name}")
                    nc.vector.stream_shuffle(xsw[:], psb[:], SWAP_MASK)
                    u = mtmp.tile([P, FREE], fp16, tag="u",
                                  name=f"u_{name}")
                    nc.vector.tensor_tensor(u[:], xsw[:], sinm[:], ALU.mult)
                    out = persist.tile([P, FREE], fp16, tag=f"rope_{name}",
                                       name=f"r{rep}_{name}")
                    nc.vector.tensor_tensor(out[:], t1[:], u[:], ALU.add)
                    return out

                def project_rope_chunk(tin, p_sb, sinm, cosm, pair, name):
                    """One [P, FREE] rope'd projection tile for one pair."""
                    ps = mmps.tile([P, FREE], f32, tag="mm")
                    for t in range(DT):
                        nc.tensor.matmul(
                            ps[:], p_sb[:, t, pair * P:(pair + 1) * P],
                            tin[:, t, :],
                            start=(t == 0), stop=(t == DT - 1))
                    return rope_from_ps(ps, sinm, cosm, name)

                # ============ startup: stream k/v/q per chunk ============
                kpos_src = qpos_d if shared_maps else kpos_d
                pk_sb = persist.tile([P, DT, 2 * P], fp16, tag="pmk",
                                     name=f"pm_k{rep}")
                pv_sb = persist.tile([P, DT, 2 * P], bf16, tag="pmv",
                                     name=f"pm_v{rep}")
                pq_sb = persist.tile([P, DT, 2 * P], fp16, tag="pmq",
                                     name=f"pm_q{rep}")
                # pv/pq loads are issued mid-chunk-0 below so the first
                # kT chunk isn't queued behind them
                nc.sync.dma_start(pk_sb[:], pk_d[:, :, :])
                vsb = persist.tile([P, MT, HLOC * 65], bf16, tag="vsb",
                                   name=f"vsb{rep}")
                nc.vector.memset(
                    vsb[:].rearrange("p m (h w) -> p m h w", h=HLOC)[:, :, :, 64:65],
                    1.0)

                krope = [[None] * MC4 for _ in range(PAIRS)]
                qrope = [[None] * NC4 for _ in range(PAIRS)]
                qdef_maps = {}
                for c in range(MC4):
                    sinm, cosm = make_maps_chunk(kpos_src, c, f"k{rep}_")
                    if shared_maps:
                        qsin, qcos = sinm, cosm
                    else:
                        qsin, qcos = make_maps_chunk(qpos_d, c, f"q{rep}_")
                    ktin = instream.tile([P, DT, FREE], fp16, tag="kin")
                    nc.sync.dma_start(ktin[:], kT_d[:, :, c * FREE:(c + 1) * FREE])
                    if c == 0:
                        nc.sync.dma_start(pv_sb[:], pv_d[:, :, :])
                    for pr in range(PAIRS):
                        krope[pr][c] = project_rope_chunk(
                            ktin, pk_sb, sinm, cosm, pr, f"k{pr}_{c}")
                    vtin = instream.tile([P, DT, FREE], bf16, tag="vin")
                    nc.sync.dma_start(vtin[:], vT_d[:, :, c * FREE:(c + 1) * FREE])
                    if c == 0:
                        nc.sync.dma_start(pq_sb[:], pq_d[:, :, :])
                    for mi4 in range(FREE // P):
                        mi = c * (FREE // P) + mi4
                        ps = mmps.tile([P, FREE], f32, tag="mm")
                        for t in range(DT):
                            nc.tensor.matmul(
                                ps[:, 0:2 * P], vtin[:, t, mi4 * P:(mi4 + 1) * P],
                                pv_sb[:, t, :],
                                start=(t == 0), stop=(t == DT - 1))
                        # strided copy on ACT (idle during projections)
                        nc.scalar.copy(
                            vsb[:, mi, :].rearrange("p (h w) -> p h w", h=HLOC)[:, :, 0:64],
                            ps[:, 0:2 * P].rearrange("p (h w) -> p h w", h=HLOC))
                    if shared_maps and c >= MC4 - 2:
                        # the last two q chunks are deferred into attention
                        # stages 0/1 (their extras slots are free and the
                        # exp stream leaves PE slack); chunk 2 is needed at
                        # stage 4, chunk 3 at stage 6 -- plenty of slack
                        qdef_maps[c] = (qsin, qcos)
                        continue
                    qtin = instream.tile([P, DT, FREE], fp16, tag="qin")
                    nc.sync.dma_start(qtin[:], qT_d[:, :, c * FREE:(c + 1) * FREE])
                    for pr in range(PAIRS):
                        qrope[pr][c] = project_rope_chunk(
                            qtin, pq_sb, qsin, qcos, pr, f"q{pr}_{c}")

                # P_o / ebc only needed from the first outproj
                nc.sync.dma_start(po_sb[:], po_d[:, :, :])

                if phase == "proj":
                    cons = mmps.tile([P, FREE], f32, tag="mm")
                    toks = [krope[pr][c] for pr in range(PAIRS) for c in range(MC4)]
                    toks += [qrope[pr][c] for pr in range(PAIRS) for c in range(NC4)]
                    for i, tk in enumerate(toks):
                        nc.tensor.matmul(cons[:], tk[:, 0:P], tk[:],
                                         start=(i == 0), stop=(i == len(toks) - 1))
                    cons2 = mmps.tile([P, FREE], f32, tag="mm")
                    nc.tensor.matmul(cons2[:, 0:P], vsb[:, 0, 0:P],
                                     vsb[:, 1, 0:P],
                                     start=True, stop=True)
                    stgx = stgp.tile([P, FREE], bf16, tag="stg")
                    nc.vector.tensor_copy(stgx[:], cons[:])
                    nc.vector.tensor_copy(stgx[:, 0:P], cons2[:, 0:P])
                    nc.sync.dma_start(out_d[0, 0:P, 0:FREE], stgx[:])
                    return

                # ============ attention stages with deferred norm/outproj
                stages = [(c, pr) for c in range(NC4) for pr in range(PAIRS)]

                def emit_st(c, pr, mi, tag):
                    stp = stps.tile([P, 2 * FREE], f32, tag="st",
                                    name=f"st{tag}")
                    for h in range(2):
                        hp = h * 64
                        nc.tensor.matmul(
                            stp[:, h * FREE:(h + 1) * FREE],
                            krope[pr][mi // (FREE // P)]
                                 [hp:hp + 64,
                                  (mi % (FREE // P)) * P:
                                  (mi % (FREE // P) + 1) * P],
                            qrope[pr][c][hp:hp + 64, :],
                            start=True, stop=True,
                            tile_position=(hp, 0))
                    return stp

                otn_store = {}   # c -> [otn(pair0), otn(pair1)]

                def norm_dve_a(pend, tail=False):
                    # evacuate the oT accumulators to SBUF first: frees the
                    # PSUM pots so the next stage's first AV never waits.
                    c, pr, pots, s = pend
                    ob = nrm.tile([P, 2, FREE], bf16, tag="ob")
                    if tail:  # ACT is idle in the tail; halve the chain
                        nc.scalar.copy(ob[0:65, 0, :], pots[0][:])
                    else:
                        nc.vector.tensor_copy(ob[0:65, 0, :], pots[0][:])
                    nc.vector.tensor_copy(ob[0:65, 1, :], pots[1][:])
                    rrf = nrm.tile([P, FREE], f32, tag="rrf")
                    nc.vector.reciprocal(rrf[0:1, :], ob[64:65, 0, :])
                    nc.vector.reciprocal(rrf[32:33, :], ob[64:65, 1, :])
                    rz = rzring[s % 2]
                    nc.vector.tensor_copy(rz[0:1, :], rrf[0:1, :])
                    nc.vector.tensor_copy(rz[32:33, :], rrf[32:33, :])
                    pend.append(rz)
                    pend.append(ob)

                def norm_pe_b(pend):
                    c, pr, pots, s, rz, ob = pend
                    rb = mmps.tile([P, FREE], f32, tag="mm")
                    nc.tensor.matmul(rb[:], ebc_sb[:], rz[:],
                                     start=True, stop=True)
                    ot = otnp.tile([P, FREE], bf16, tag="otn")
                    nc.vector.tensor_tensor(ot[0:64, :], ob[0:64, 0, :],
                                            rb[0:64, :], ALU.mult)
                    nc.vector.tensor_tensor(ot[64:128, :], ob[0:64, 1, :],
                                            rb[64:128, :], ALU.mult)
                    otn_store.setdefault(c, []).append(ot)

                def outproj_pieces(cc, tail=False):
                    """Thunks: matmul+stage+dma pieces, with a half-chunk
                    ReduceScatter after each group (the RS output lands in
                    out_part via a small DRAM bounce).

                    In the tail the attention score ring (2x [P,2*FREE]
                    PSUM tiles) is idle, so pieces run at free=1024: half
                    the instructions on the copy/DMA critical path."""
                    HR = FREE // 2                    # inb rows per half
                    HO = HR // group_size             # out rows per half
                    inbh = [dramp.tile([HR, d], bf16, tag="inb",
                                       name=f"inb{rep}_{cc}_{hh}")
                            for hh in range(2)]

                    def piece(nt, dc):
                        def go():
                            otns = otn_store[cc]
                            ops_ = mmps.tile([P, FREE], f32, tag="mm")
                            for t in range(PAIRS):
                                nc.tensor.matmul(
                                    ops_[:], otns[t][:, nt * P:(nt + 1) * P],
                                    po_sb[:, t, dc * FREE:(dc + 1) * FREE],
                                    start=(t == 0), stop=(t == PAIRS - 1))
                            stg = stgp.tile([P, FREE], bf16, tag="stg")
                            nc.vector.tensor_copy(stg[:], ops_[:])
                            nc.sync.dma_start(
                                inbh[nt // 2][(nt % 2) * P:(nt % 2 + 1) * P,
                                              dc * FREE:(dc + 1) * FREE],
                                stg[:])
                        return go

                    def wide_piece(nt):
                        def go():
                            otns = otn_store[cc]
                            ops_ = stps.tile([P, 2 * FREE], f32, tag="st",
                                             name=f"tw{rep}_{nt}")
                            # a matmul may only write one PSUM bank (512
                            # f32 cols): two half-matmuls, one wide copy
                            for dc in range(DC):
                                for t in range(PAIRS):
                                    nc.tensor.matmul(
                                        ops_[:, dc * FREE:(dc + 1) * FREE],
                                        otns[t][:, nt * P:(nt + 1) * P],
                                        po_sb[:, t, dc * FREE:(dc + 1) * FREE],
                                        start=(t == 0), stop=(t == PAIRS - 1))
                            stg = stgp.tile([P, 2 * FREE], bf16, tag="stgw",
                                            name=f"tws{rep}_{nt}")
                            if nt % 2 == 0:  # alternate ACT/DVE copies
                                nc.scalar.copy(stg[:], ops_[:])
                            else:
                                nc.vector.tensor_copy(stg[:], ops_[:])
                            nc.sync.dma_start(
                                inbh[nt // 2][(nt % 2) * P:(nt % 2 + 1) * P, :],
                                stg[:])
                        return go

                    def rs_half(h):
                        def go():
                            if use_collective:
                                outb = dramp.tile([HO, d], bf16, tag="outb",
                                                  name=f"outb{rep}_{cc}_{h}")
                                nc.gpsimd.collective_compute(
                                    "ReduceScatter", mybir.AluOpType.add,
                                    replica_groups=RG,
                                    ins=[inbh[h].opt()], outs=[outb.opt()])
                                nc.sync.dma_start(
                                    out_d[cc, h * HO:(h + 1) * HO, :],
                                    outb[:, :])
                            else:
                                nc.sync.dma_start(
                                    out_d[cc, h * HR:(h + 1) * HR, :],
                                    inbh[h][:, :])
                        return go

                    thunks = []
                    for nt in range(NTPC):
                        if tail:
                            thunks.append(wide_piece(nt))
                        else:
                            for dc in range(DC):
                                thunks.append(piece(nt, dc))
                        if nt == NTPC // 2 - 1:
                            thunks.append(rs_half(0))
                    thunks.append(rs_half(1))
                    return thunks

                def emit_av(av):
                    pot, pr, mi, ex = av
                    for h in range(2):
                        hc = (2 * pr + h) * 65
                        nc.tensor.matmul(
                            pot[h][:], vsb[:, mi, hc:hc + 65],
                            ex[:, h * FREE:(h + 1) * FREE],
                            start=(mi == 0), stop=(mi == MT - 1))

                # oT accumulation lags one iteration behind exp so the PE
                # work gated on exp(mi-1) starts with ST(mi+1) -- the exp
                # stream then never waits behind oT + deferred work.
                pending = None       # [c, pr, pots, stage_idx(, rz, ob)]
                piece_queue = []     # deferred outproj thunks
                av_pending = None    # (pot, pr, mi, ex) to emit next iter
                for s, (c, pr) in enumerate(stages):
                    extras = {}
                    if pending is not None:
                        pc = pending[0]
                        extras[0] = [lambda p=pending: norm_dve_a(p)]
                        extras[1] = [lambda p=pending: norm_pe_b(p)]
                        if pr == 0:  # previous stage closed chunk c-1
                            piece_queue.extend(outproj_pieces(pc))
                    if s in (0, 1) and shared_maps:
                        # deferred projection+rope of q chunk 3 (stage 0) /
                        # chunk 2 (stage 1), 2 accumulation steps per iter
                        cq = MC4 - 1 - s
                        qsd, qcd = qdef_maps[cq]
                        qtind = instream.tile([P, DT, FREE], fp16,
                                              tag="qin", name=f"qtind{s}")
                        nc.sync.dma_start(
                            qtind[:], qT_d[:, :, cq * FREE:(cq + 1) * FREE])

                        def qd_steps(prr, t0, hold, tin_, cq_):
                            def go():
                                if t0 == 0:
                                    hold["ps"] = mmps.tile(
                                        [P, FREE], f32, tag="mm",
                                        name=f"qdps{prr}_{cq_}")
                                for t in (t0, t0 + 1):
                                    nc.tensor.matmul(
                                        hold["ps"][:],
                                        pq_sb[:, t, prr * P:(prr + 1) * P],
                                        tin_[:, t, :],
                                        start=(t == 0), stop=(t == DT - 1))
                            return go

                        def qd_rope(prr, hold, sm, cm, cq_):
                            def go():
                                qrope[prr][cq_] = rope_from_ps(
                                    hold["ps"], sm, cm, f"q{prr}_{cq_}")
                            return go

                        for prr in range(PAIRS):
                            hold = {}
                            base = 2 + prr * 5
                            for i, t0 in enumerate(range(0, DT, 2)):
                                extras.setdefault(base + i, []).append(
                                    qd_steps(prr, t0, hold, qtind, cq))
                            extras.setdefault(base + 4, []).append(
                                qd_rope(prr, hold, qsd, qcd, cq))
                    slot = 6
                    while piece_queue and slot < MT:
                        extras.setdefault(slot, []).append(piece_queue.pop(0))
                        slot += 2
                    pot = [otps.tile([65, FREE], f32, tag="ot",
                                     name=f"ot{rep}_{c}_{pr}_{hh}")
                           for hh in range(2)]
                    stp = emit_st(c, pr, 0, f"{rep}_{c}_{pr}_0")
                    for mi in range(MT):
                        stp_next = (emit_st(c, pr, mi + 1,
                                            f"{rep}_{c}_{pr}_{mi+1}")
                                    if mi + 1 < MT else None)
                        ex = expp.tile([P, 2 * FREE], bf16, tag="exp")
                        nc.scalar.activation(ex[:], stp[:], AF.Exp)
                        if av_pending is not None:
                            emit_av(av_pending)
                        av_pending = (pot, pr, mi, ex)
                        for th in extras.get(mi, []):
                            th()
                        stp = stp_next
                    pending = [c, pr, pot, s]

                # tail: last AV + last stage's norm + last outproj + RS
                emit_av(av_pending)
                norm_dve_a(pending, tail=True)
                norm_pe_b(pending)
                for th in piece_queue:
                    th()
                for th in outproj_pieces(NC4 - 1, tail=True):
                    th()

            for rep in range(reps):
                if rep:
                    tc.strict_bb_all_engine_barrier()
                one_pass(rep)

    nc.compile()
    return nc


# ------------------------------------------------------------------- host

def _prep_core_inputs(query, q_positions, key, k_positions, value,
                      P_q, P_k, P_v, P_o, core, n=N, m=M, d=D, _cache=None):
    """Build the per-core input map (numpy, host-side shard/layout prep).

    With _cache (a dict shared across calls), per-batch activation shards
    and per-head-group weight packs are computed once and shared by
    reference across the 4 (resp. 2) cores that use them."""
    import ml_dtypes
    bf16 = ml_dtypes.bfloat16
    b = core // 4
    g = core % 4
    DT = d // P
    hsl = slice(g * HLOC, (g + 1) * HLOC)

    def t_in(x, length):  # [length, d] -> [P, DT, length] bf16
        return np.ascontiguousarray(
            x.T.reshape(DT, P, length).transpose(1, 0, 2).astype(bf16))

    def t_in16(x, length):  # [length, d] -> [P, DT, length] fp16
        return np.ascontiguousarray(
            x.T.reshape(DT, P, length).transpose(1, 0, 2).astype(np.float16))

    def memo(key_, fn):
        if _cache is None:
            return fn()
        if key_ not in _cache:
            _cache[key_] = fn()
        return _cache[key_]

    # interleaved k-dim order: stationary col c (per head) holds original
    # k index (c%2)*32 + c//2, so the rope partner sits on the adjacent
    # partition (stream_shuffle-able swap).
    KPERM = np.array([(c % 2) * 32 + c // 2 for c in range(64)])

    def pack_pqk(Pm, dtype=None):
        # [HLOC, d, 64] -> [P, DT, 2*P] head-pair stationaries
        out = np.empty((P, DT, 2 * P), np.float32)
        for p in range(PAIRS):
            for hl in range(2):
                h = 2 * p + hl
                out[:, :, p * P + hl * 64: p * P + hl * 64 + 64] = \
                    Pm[h].reshape(DT, P, 64).transpose(1, 0, 2)[:, :, KPERM]
        return np.ascontiguousarray(out.astype(dtype or bf16))

    def pack_pv(Pm):  # [HLOC, d, 64] -> [P, DT, 256] (hv on free)
        return np.ascontiguousarray(
            Pm.reshape(HLOC, DT, P, 64).transpose(2, 1, 0, 3)
            .reshape(P, DT, 2 * P).astype(bf16))

    def pack_po(Pm):  # [HLOC, d, V] -> [P, PAIRS, d];  hv = t*128 + p
        out = np.empty((P, PAIRS, d), np.float32)
        for t in range(PAIRS):
            for hl in range(2):
                h = 2 * t + hl
                out[hl * 64:(hl + 1) * 64, t, :] = Pm[h].T  # [V, d]
        return np.ascontiguousarray(out.astype(bf16))

    jj = np.arange(P) % 64
    j_idx = jj // 2          # timescale index in interleaved layout
    half = jj % 2            # 0 -> x1 rows (get -sin), 1 -> x2 rows (+sin)
    frac = 2.0 * j_idx.astype(np.float32) / 64.0
    invt = (np.float32(MAX_WAVELENGTH) ** (-frac)).astype(np.float32) / np.float32(SCALE_FACTOR)
    sign = np.where(half == 0, -1.0, 1.0).astype(np.float32)
    two_pi = np.float32(2.0 * math.pi)
    rc = np.stack([invt / two_pi, sign * two_pi], axis=1).astype(np.float32)

    def pos_bcast(pos):  # [L] -> [P, L] fp16 (ints <= 2048 exact)
        return np.ascontiguousarray(
            np.broadcast_to(pos.astype(np.float16), (P, pos.shape[0])))

    return {
        "qT": memo(("qT", b), lambda: t_in16(query[b], n)),
        "kT": memo(("kT", b), lambda: t_in16(key[b], m)),
        "vT": memo(("vT", b), lambda: t_in(value[b], m)),
        "pq": memo(("pq", g), lambda: pack_pqk(P_q[hsl], np.float16)),
        "pk": memo(("pk", g), lambda: pack_pqk(P_k[hsl], np.float16)),
        "pv": memo(("pv", g), lambda: pack_pv(P_v[hsl])),
        "po": memo(("po", g), lambda: pack_po(P_o[hsl])),
        "qposb": memo(("qposb", b), lambda: pos_bcast(q_positions[b])),
        "kposb": memo(("kposb", b), lambda: pos_bcast(k_positions[b])),
        "ropec": memo("ropec", lambda: rc),
    }


def assemble_output(results, n=N, d=D, group_size=4):
    """Gather per-core [NC4, 128, d] RS shards into the full [B, n, d].

    Each chunk is ReduceScattered in two halves: rank r of a group holds
    rows [h*256 + r*64, ...+64) of chunk c in its out_part[c, h*64:...]."""
    NC4 = n // FREE
    HR = FREE // 2
    HO = HR // group_size
    out = np.empty((B, n, d), np.float32)
    for core in range(N_CORES):
        b, r = core // group_size, core % group_size
        part = np.asarray(results[core]["out_part"]).astype(np.float32) \
                 .reshape(NC4, 2 * HO, d)
        for c in range(NC4):
            for h in range(2):
                g0 = c * FREE + h * HR + r * HO
                out[b, g0:g0 + HO, :] = part[c, h * HO:(h + 1) * HO]
    return out


def kernel(query, q_positions, key, k_positions, value, mask=None,
           P_q=None, P_k=None, P_v=None, P_o=None, **_unused):
    from concourse.bass_utils import run_bass_kernel_spmd

    query = np.asarray(query, np.float32)
    key = np.asarray(key, np.float32)
    value = np.asarray(value, np.float32)
    q_positions = np.asarray(q_positions, np.int32)
    k_positions = np.asarray(k_positions, np.int32)
    P_q = np.asarray(P_q, np.float32)
    P_k = np.asarray(P_k, np.float32)
    P_v = np.asarray(P_v, np.float32)
    P_o = np.asarray(P_o, np.float32)

    shared = (N == M) and np.array_equal(q_positions, k_positions)
    key_dims = (N, M, D, shared)
    if key_dims not in _COMPILED:
        _COMPILED[key_dims] = build_nc(N, M, D, shared_maps=shared)
    nc = _COMPILED[key_dims]

    cache = {}
    in_maps = [
        _prep_core_inputs(query, q_positions, key, k_positions, value,
                          P_q, P_k, P_v, P_o, core, _cache=cache)
        for core in range(N_CORES)
    ]
    res = run_bass_kernel_spmd(nc, in_maps, list(range(N_CORES)))
    return assemble_output(res.results)


if __name__ == "__main__":
    print("building...")
    build_nc()
    print("ok")
